# revision 16
# baseline (speedup 1.0000x reference)
"""MoE feed-forward (RMSNorm -> top-2 router -> SwiGLU experts -> combine)
on 8 TRN2 NeuronCores, data-parallel over tokens with all weights replicated.

v3: phase-A restructured (host-side x^T kills all router transposes+copies,
norm sum-of-squares on DVE, batched per-4-tile-group top-2 + grouping with
one prefix matmul, gates deferred to one batched sigmoid), Silu activation
(halves SwiGLU DVE work), effective capacity 576 (vs 640 storage; data max
568), fused scalar_tensor_tensor combine with bf16 skip kept in SBUF,
batched 2-offset combine gathers.

Per core (2048 tokens):
  - norm factor s/4 via DVE tensor_tensor_reduce; xn8 = x*(s/4) fp8
  - router scores f32 via PE with lhsT = x^T chunks (host layout)
  - per 4-tile group: batched top-2 + capacity-grouped permutation
    (rank matmul over 8 blocks, count matmul, prefix matmul vs Tsel)
  - xn8 rows scattered (indirect DMA) into per-expert groups in DRAM
  - per expert: gather fp8 rows, PE-transpose, fp8 DoubleRow up-GEMM
    (h^T layout), Silu+mult SwiGLU, fp8 DR down-GEMM, f16 writes
  - combine: batched gather both expert rows, (g0*w0+skip)+(g1*w1) STT

Self-contained: hardcodes all shapes; no file reads.
"""
import numpy as np
import ml_dtypes

T_PER_CORE = 2048
D = 1024
H = 2048
E = 8
N_CORES = 8
CCAP = 640   # per-(core,expert) storage capacity (row pitch)
CEFF = 576   # computed rows per group; actual seed-0 max count is 568
EPS = 1e-6
SW = 4.0     # weight scale folded into fp8 up-weights; acts carry 1/SW

NT = T_PER_CORE // 128   # 16 token tiles
NG = NT // 4             # 4 groups of 4 tiles
NB = 2 * NT              # 32 scatter blocks

_CACHE = {}


def _split_excess_waits(nc, max_waits=1):
    """walrus in this env caps sync-wait commands per instruction; move excess
    waits onto same-engine NOPs inserted just before the instruction."""
    import concourse.mybir as mybir

    n_split = 0
    for fn in nc.m.functions:
        for blk in fn.blocks:
            new_list = []
            for inst in blk.instructions:
                si = getattr(inst, "sync_info", None)
                waits = list(si.on_wait) if si is not None and si.on_wait else []
                if len(waits) > max_waits:
                    n_split += 1
                    excess = waits[: len(waits) - max_waits]
                    si.on_wait = waits[len(waits) - max_waits:]
                    for ci in range(0, len(excess), max_waits):
                        new_list.append(
                            mybir.InstNoOp(
                                name=f"waitsplit-{n_split}-{ci}",
                                engine=inst.engine,
                                ins=[],
                                outs=[],
                                sync_info=mybir.SyncInfo(
                                    on_wait=excess[ci: ci + max_waits], on_update=[]
                                ),
                            )
                        )
                new_list.append(inst)
            blk.instructions = new_list
    return n_split


def build_program(ceff=CEFF, split_waits=True, use_silu=True, batched_gather=True):
    import concourse.bass as bass
    import concourse.mybir as mybir
    import concourse.tile as tile

    f32 = mybir.dt.float32
    f16 = mybir.dt.float16
    bf16 = mybir.dt.bfloat16
    f8 = mybir.dt.float8e4
    i32 = mybir.dt.int32
    u8 = mybir.dt.uint8
    AF = mybir.ActivationFunctionType
    OP = mybir.AluOpType
    AX = mybir.AxisListType
    DR = mybir.MatmulPerfMode.DoubleRow

    CT = ceff - 512          # tail width (64)
    RT = (ceff + 127) // 128  # 5 gather/row tiles per expert (last partial)

    nc = bass.Bass()

    xbf_d = nc.declare_dram_parameter("xbf", [T_PER_CORE, D], bf16, isOutput=False)
    xT_d = nc.declare_dram_parameter("xT", [128, 8 * T_PER_CORE], f32, isOutput=False)
    wr_d = nc.declare_dram_parameter("wr", [128, 8 * E], f32, isOutput=False)
    wu_d = nc.declare_dram_parameter("wu", [E, 128, 8 * 2 * H], f8, isOutput=False)
    wd_d = nc.declare_dram_parameter("wd", [E, 128, 8 * 2 * D], f8, isOutput=False)
    ident8_d = nc.declare_dram_parameter("ident8", [128, 128], f8, isOutput=False)
    cum_d = nc.declare_dram_parameter("cum", [128, 128], bf16, isOutput=False)
    tsel_d = nc.declare_dram_parameter("tsel", [64, 72], f32, isOutput=False)
    iota_ie_d = nc.declare_dram_parameter("iota_ie", [128, 32], f32, isOutput=False)
    iota_ei_d = nc.declare_dram_parameter("iota_ei", [128, 32], f32, isOutput=False)
    base8_d = nc.declare_dram_parameter("base8", [1, E], f32, isOutput=False)
    out_d = nc.declare_dram_parameter("out", [T_PER_CORE, D], f32, isOutput=True)

    g_dram = nc.dram_tensor("g_dram", [E * CCAP, D], f8)
    dn_dram = nc.dram_tensor("dn_dram", [E * CCAP, D], f16)

    with tile.TileContext(nc) as tc:
        with (
            tc.tile_pool(name="consts", bufs=1) as pc,
            tc.tile_pool(name="longl", bufs=1) as pl,
        ):
            ident8_sb = pc.tile_from(ident8_d[:])
            cum_sb = pc.tile_from(cum_d[:])
            tsel_sb = pc.tile_from(tsel_d[:])
            iota_ie_sb = pc.tile_from(iota_ie_d[:])   # [128, 4, 8] views
            iota_ei_sb = pc.tile_from(iota_ei_d[:])   # [128, 8, 4] views
            wr_sb = pc.tile_from(wr_d[:])             # [128, 8, 8] views
            big_sb = pc.tile([128, 32], f32)
            nc.vector.memset(big_sb[:], 1e9)
            neg_sb = pc.tile([128, 32], f32)
            nc.vector.memset(neg_sb[:], -1e30)
            zeros64_sb = pc.tile([128, 64], f32)
            nc.vector.memset(zeros64_sb[:], 0.0)
            onesb1 = pc.tile([1, 128], f32)
            nc.vector.memset(onesb1[:], 1.0)
            ones128bf = pc.tile([128, 1], bf16)
            nc.vector.memset(ones128bf[:], 1.0)
            ones_col = pc.tile([128, 1], f32)
            nc.vector.memset(ones_col[:], 1.0)
            epsb_col = pc.tile([128, 1], f32)
            nc.vector.memset(epsb_col[:], EPS * SW * SW)

            s_all = pl.tile([128, NT], f32)
            ms_all = pl.tile([128, NT], f32)
            sd_all = pl.tile([128, NT], f32)
            gap_all = pl.tile([128, NT], f32)
            w0_all = pl.tile([128, NT], f32)
            w1_all = pl.tile([128, NT], f32)
            # dest_all layout: [p, tile*2 + k] (pairs adjacent for phase F)
            dest_all = pl.tile([128, NB], i32)
            xbf_all = pl.tile([128, NT, D], bf16)
            xn8_all = pl.tile([128, NT, D], f8)

            # ---------------- Phase A: norm, router, top-2, grouping, scatter
            with (
                tc.tile_pool(name="pxT", bufs=2) as pxT,
                tc.tile_pool(name="pscr", bufs=2) as pscr,
                tc.tile_pool(name="psmA", bufs=6) as psmA,
                tc.tile_pool(name="pog", bufs=2) as pog,
                tc.tile_pool(name="poffs", bufs=2) as poffs,
                tc.tile_pool(name="prun", bufs=2) as prun,
                tc.tile_pool(name="psc", bufs=2, space="PSUM") as psc,
                tc.tile_pool(name="ppos", bufs=2, space="PSUM") as ppos,
                tc.tile_pool(name="pcnt", bufs=2, space="PSUM") as pcnt,
            ):
                run_row = prun.tile([1, E], f32, tag="run")
                nc.sync.dma_start(out=run_row[:], in_=base8_d[:])
                xT_view = xT_d[:].rearrange("p (dc t) -> p dc t", dc=8)
                for g in range(NG):
                    g4 = slice(4 * g, 4 * g + 4)
                    xT_g = pxT.tile([128, 8, 512], f32, tag="xT")
                    nc.sync.dma_start(
                        out=xT_g[:], in_=xT_view[:, :, g * 512:(g + 1) * 512]
                    )
                    scp = psc.tile([128, 4, E], f32, tag="sc")
                    for il in range(4):
                        i = 4 * g + il
                        ts = slice(i * 128, (i + 1) * 128)
                        nc.sync.dma_start(out=xbf_all[:, i, :], in_=xbf_d[ts, :])
                        scr = pscr.tile([128, D], bf16, tag="scr")
                        nc.vector.tensor_tensor(
                            scr[:], xbf_all[:, i, :], xbf_all[:, i, :], op=OP.mult
                        )
                        nc.vector.tensor_reduce(
                            ms_all[:, i: i + 1], scr[:], axis=AX.X, op=OP.add
                        )
                        for dc in range(8):
                            nc.tensor.matmul(
                                out=scp[:, il, :],
                                lhsT=xT_g[:, dc, il * 128:(il + 1) * 128],
                                rhs=wr_sb[:].rearrange("p (dc e) -> p dc e", dc=8)[
                                    :, dc, :
                                ],
                                start=(dc == 0), stop=(dc == 7),
                                skip_group_check=True,
                            )

                    # norm factors for the group
                    nc.scalar.activation(
                        sd_all[:, g4], ms_all[:, g4], AF.Sqrt,
                        bias=epsb_col[:], scale=SW * SW / D,
                    )
                    nc.vector.reciprocal(s_all[:, g4], sd_all[:, g4])
                    for il in range(4):
                        i = 4 * g + il
                        nc.vector.tensor_scalar_mul(
                            xn8_all[:, i, :], xbf_all[:, i, :], s_all[:, i: i + 1]
                        )

                    # batched top-2 on [128, 4, 8]
                    s_sb = psmA.tile([128, 4, E], f32, tag="ssb")
                    nc.vector.tensor_copy(s_sb[:], scp[:])
                    iota_ie = iota_ie_sb[:].rearrange("p (i e) -> p i e", i=4)
                    iota_ei = iota_ei_sb[:].rearrange("p (e i) -> p e i", e=8)
                    big_ie = big_sb[:].rearrange("p (i e) -> p i e", i=4)
                    neg_ie = neg_sb[:].rearrange("p (i e) -> p i e", i=4)
                    m0 = psmA.tile([128, 4], f32, tag="m0")
                    nc.vector.tensor_reduce(m0[:], s_sb[:], axis=AX.X, op=OP.max)
                    eq0 = psmA.tile([128, 4, E], u8, tag="eq")
                    nc.vector.tensor_tensor(
                        eq0[:], s_sb[:], m0[:].unsqueeze(2).to_broadcast([128, 4, E]),
                        op=OP.is_equal,
                    )
                    cand = psmA.tile([128, 4, E], f32, tag="cand")
                    nc.vector.select(cand[:], eq0[:], iota_ie, big_ie)
                    i0f = psmA.tile([128, 4], f32, tag="i0")
                    nc.vector.tensor_reduce(i0f[:], cand[:], axis=AX.X, op=OP.min)
                    eqA = psmA.tile([128, 4, E], u8, tag="eq")
                    nc.vector.tensor_tensor(
                        eqA[:], iota_ie, i0f[:].unsqueeze(2).to_broadcast([128, 4, E]),
                        op=OP.is_equal,
                    )
                    sc2 = psmA.tile([128, 4, E], f32, tag="sc2")
                    nc.vector.select(sc2[:], eqA[:], neg_ie, s_sb[:])
                    m1 = psmA.tile([128, 4], f32, tag="m1")
                    nc.vector.tensor_reduce(m1[:], sc2[:], axis=AX.X, op=OP.max)
                    eq1 = psmA.tile([128, 4, E], u8, tag="eq")
                    nc.vector.tensor_tensor(
                        eq1[:], sc2[:], m1[:].unsqueeze(2).to_broadcast([128, 4, E]),
                        op=OP.is_equal,
                    )
                    cand1 = psmA.tile([128, 4, E], f32, tag="cand")
                    nc.vector.select(cand1[:], eq1[:], iota_ie, big_ie)
                    i1f = psmA.tile([128, 4], f32, tag="i1")
                    nc.vector.tensor_reduce(i1f[:], cand1[:], axis=AX.X, op=OP.min)

                    # gates (sigmoid deferred): gap = (m0-m1)*s
                    nc.vector.tensor_sub(gap_all[:, g4], m0[:], m1[:])
                    nc.vector.tensor_tensor(
                        gap_all[:, g4], gap_all[:, g4], s_all[:, g4], op=OP.mult
                    )

                    # one-hots in (e, block) layout; block b = k*4 + il
                    ohu_g = pog.tile([128, E, 8], u8, tag="ohu")
                    nc.vector.tensor_tensor(
                        ohu_g[:, :, 0:4], iota_ei,
                        i0f[:].unsqueeze(1).to_broadcast([128, E, 4]),
                        op=OP.is_equal,
                    )
                    nc.vector.tensor_tensor(
                        ohu_g[:, :, 4:8], iota_ei,
                        i1f[:].unsqueeze(1).to_broadcast([128, E, 4]),
                        op=OP.is_equal,
                    )
                    oh_g = pog.tile([128, E, 8], bf16, tag="oh")
                    nc.vector.tensor_copy(oh_g[:], ohu_g[:])

                    # counts per (e,b): [64,1] = oh^T @ ones
                    oh_flat = oh_g[:].rearrange("p e b -> p (e b)")
                    cntp = pcnt.tile([64, 1], f32, tag="cnt")
                    nc.tensor.matmul(
                        out=cntp[:], lhsT=oh_flat, rhs=ones128bf[:],
                        start=True, stop=True,
                    )
                    cnt_sb = psmA.tile([64, 1], f32, tag="cntsb")
                    nc.vector.tensor_copy(cnt_sb[:], cntp[:])
                    # intra-group exclusive prefix over blocks (col 8: totals)
                    offp = pcnt.tile([1, 72], f32, tag="off")
                    nc.tensor.matmul(
                        out=offp[:], lhsT=cnt_sb[:], rhs=tsel_sb[:],
                        start=True, stop=True,
                    )
                    offs_sb = poffs.tile([1, E, 9], f32, tag="offs")
                    nc.vector.tensor_tensor(
                        offs_sb[:], offp[:].rearrange("p (e b) -> p e b", e=8),
                        run_row[:].unsqueeze(2).to_broadcast([1, E, 9]),
                        op=OP.add,
                    )
                    run_next = prun.tile([1, E], f32, tag="run")
                    nc.vector.tensor_copy(
                        run_next[:].unsqueeze(2), offs_sb[:, :, 8:9]
                    )
                    run_row = run_next

                    # dest = rank-in-block + group offset, per (token, e, b)
                    pos = ppos.tile([128, E, 8], f32, tag="pos")
                    nc.tensor.matmul(
                        out=pos[:].rearrange("p e b -> p (e b)"), lhsT=cum_sb[:],
                        rhs=oh_flat, start=True, stop=False, skip_group_check=True,
                    )
                    nc.tensor.matmul(
                        out=pos[:].rearrange("p e b -> p (e b)"), lhsT=onesb1[:],
                        rhs=offs_sb[:, :, 0:8], start=False, stop=True,
                        skip_group_check=True,
                    )
                    seld = psmA.tile([128, E, 8], f32, tag="seld")
                    nc.vector.select(
                        seld[:], ohu_g[:], pos[:],
                        zeros64_sb[:].rearrange("p (e b) -> p e b", e=8),
                    )
                    destf = psmA.tile([128, 8], f32, tag="destf")
                    nc.vector.tensor_reduce(
                        destf[:], seld[:].transpose([0, 2, 1]), axis=AX.X, op=OP.add
                    )
                    dest_tk = dest_all[:].rearrange("p (t k) -> p t k", k=2)
                    nc.vector.tensor_copy(
                        dest_tk[:, 4 * g: 4 * g + 4, 0:1],
                        destf[:, 0:4].unsqueeze(2),
                    )
                    nc.vector.tensor_copy(
                        dest_tk[:, 4 * g: 4 * g + 4, 1:2],
                        destf[:, 4:8].unsqueeze(2),
                    )
                    for b in range(8):
                        col = (4 * g + (b % 4)) * 2 + (b // 4)
                        nc.gpsimd.indirect_dma_start(
                            out=g_dram[:],
                            out_offset=bass.IndirectOffsetOnAxis(
                                ap=dest_all[:, col: col + 1], axis=0
                            ),
                            in_=xn8_all[:, 4 * g + (b % 4), :],
                            in_offset=None,
                        )

                # batched gates
                nc.scalar.activation(w0_all[:], gap_all[:], AF.Sigmoid, scale=SW)
                nc.vector.tensor_tensor(
                    w1_all[:], ones_col[:].to_broadcast([128, NT]), w0_all[:],
                    op=OP.subtract,
                )

            # ---------------- Phase E: expert FFN loop
            with (
                tc.tile_pool(name="pgr", bufs=3) as pgr,
                tc.tile_pool(name="pgts", bufs=2) as pgts,
                tc.tile_pool(name="pwu", bufs=2) as pwu,
                tc.tile_pool(name="pwd", bufs=2) as pwd,
                tc.tile_pool(name="pht", bufs=1) as pht,
                tc.tile_pool(name="psil", bufs=2) as psil,
                tc.tile_pool(name="psilt", bufs=2) as psilt,
                tc.tile_pool(name="pdo", bufs=3) as pdo,
                tc.tile_pool(name="ppmain", bufs=2, space="PSUM") as ppmain,
                tc.tile_pool(name="pptail", bufs=2, space="PSUM") as pptail,
                tc.tile_pool(name="ppd", bufs=2, space="PSUM") as ppd,
            ):
                for e in range(E):
                    wu_sb = pwu.tile([128, 4, 2, 2 * H], f8, tag="wu")
                    nc.sync.dma_start(
                        out=wu_sb[:].rearrange("p dp kt c -> p (dp kt c)"),
                        in_=wu_d[e],
                    )
                    wd_sb = pwd.tile([128, 8, 2, D], f8, tag="wd")
                    nc.sync.dma_start(
                        out=wd_sb[:].rearrange("p q kt d -> p (q kt d)"), in_=wd_d[e]
                    )

                    # gather fp8 rows + transpose to [d, rows]
                    gts = pgts.tile([128, 8, ceff], f8, tag="gts")
                    for rt in range(RT):
                        w = min(128, ceff - rt * 128)
                        gr = pgr.tile([128, D], f8, tag="gr")
                        nc.sync.dma_start(
                            out=gr[:],
                            in_=g_dram[
                                e * CCAP + rt * 128: e * CCAP + (rt + 1) * 128, :
                            ],
                        )
                        for half in range(2):
                            # fp8 transpose hw mode writes with element step 2
                            trb = ppmain.tile([128, 4, 128, 2], f8, tag="pug")
                            for q in range(4):
                                dc = half * 4 + q
                                nc.tensor.transpose(
                                    trb[:, q, :, 0],
                                    gr[:, dc * 128:(dc + 1) * 128],
                                    ident8_sb[:],
                                )
                            nc.vector.tensor_copy(
                                gts[:, half * 4:(half + 1) * 4,
                                    rt * 128: rt * 128 + w],
                                trb[:, :, 0:w, 0],
                            )

                    # up-GEMM (fp8 DoubleRow, h^T layout) + SwiGLU via Silu
                    hts = pht.tile([128, 16, ceff], f8, tag="ht")
                    for hg in range(16):
                        pug = ppmain.tile([128, 1024], f32, tag="pug")
                        pugt = pptail.tile([128, 2 * CT], f32, tag="pugt")
                        for part in range(2):
                            col0 = hg * 128 + part * H
                            for dp in range(4):
                                nc.tensor.matmul(
                                    out=pug[:, part * 512:(part + 1) * 512],
                                    lhsT=wu_sb[:, dp, :, col0:col0 + 128],
                                    rhs=gts[:, 2 * dp:2 * dp + 2, 0:512],
                                    start=(dp == 0), stop=(dp == 3), perf_mode=DR,
                                )
                            for dp in range(4):
                                nc.tensor.matmul(
                                    out=pugt[:, part * CT:(part + 1) * CT],
                                    lhsT=wu_sb[:, dp, :, col0:col0 + 128],
                                    rhs=gts[:, 2 * dp:2 * dp + 2, 512:ceff],
                                    start=(dp == 0), stop=(dp == 3), perf_mode=DR,
                                )
                        for (pt, po, cs, w, pp) in (
                            (pug, 512, slice(0, 512), 512, psil),
                            (pugt, CT, slice(512, ceff), CT, psilt),
                        ):
                            sil = pp.tile([128, w], f32, tag=f"sil{w}")
                            if use_silu:
                                nc.scalar.activation(
                                    sil[:], pt[:, po:2 * po], AF.Silu
                                )
                                nc.vector.tensor_tensor(
                                    hts[:, hg, cs], pt[:, 0:po], sil[:],
                                    op=OP.mult,
                                )
                            else:
                                nc.scalar.activation(
                                    sil[:], pt[:, po:2 * po], AF.Sigmoid
                                )
                                h1 = pp.tile([128, w], f32, tag=f"h1{w}")
                                nc.vector.tensor_tensor(
                                    h1[:], pt[:, 0:po], sil[:], op=OP.mult
                                )
                                nc.vector.tensor_tensor(
                                    hts[:, hg, cs], h1[:], pt[:, po:2 * po],
                                    op=OP.mult,
                                )

                    # down-GEMM (fp8 DR)
                    for rt in range(RT):
                        m = min(128, ceff - rt * 128)
                        rs = slice(rt * 128, rt * 128 + m)
                        do = pdo.tile([128, D], f16, tag="do")
                        for dq in range(2):
                            pd = ppd.tile([128, 512], f32, tag="pd")
                            for q in range(8):
                                nc.tensor.matmul(
                                    out=pd[0:m, :],
                                    lhsT=hts[:, 2 * q:2 * q + 2, rs],
                                    rhs=wd_sb[:, q, :, dq * 512:(dq + 1) * 512],
                                    start=(q == 0), stop=(q == 7), perf_mode=DR,
                                )
                            nc.vector.tensor_copy(
                                do[0:m, dq * 512:(dq + 1) * 512], pd[0:m, :]
                            )
                        nc.sync.dma_start(
                            out=dn_dram[e * CCAP + rt * 128: e * CCAP + rt * 128 + m, :],
                            in_=do[0:m, :],
                        )

            # ---------------- Phase F: combine
            with (
                tc.tile_pool(name="pgd", bufs=6) as pgd,
                tc.tile_pool(name="pcmb", bufs=3) as pcmb,
            ):
                for i in range(NT):
                    ts = slice(i * 128, (i + 1) * 128)
                    g01 = pgd.tile([128, 2, D], f16, tag="gd")
                    if batched_gather:
                        nc.gpsimd.indirect_dma_start(
                            out=g01[:],
                            out_offset=None,
                            in_=dn_dram[:],
                            in_offset=bass.IndirectOffsetOnAxis(
                                ap=dest_all[:, 2 * i: 2 * i + 2], axis=0
                            ),
                        )
                    else:
                        for k in range(2):
                            nc.gpsimd.indirect_dma_start(
                                out=g01[:, k, :],
                                out_offset=None,
                                in_=dn_dram[:],
                                in_offset=bass.IndirectOffsetOnAxis(
                                    ap=dest_all[:, 2 * i + k: 2 * i + k + 1],
                                    axis=0,
                                ),
                            )
                    acc = pcmb.tile([128, D], f32, tag="acc")
                    nc.vector.scalar_tensor_tensor(
                        out=acc[:], in0=g01[:, 0, :], scalar=w0_all[:, i: i + 1],
                        in1=xbf_all[:, i, :], op0=OP.mult, op1=OP.add,
                    )
                    outt = pcmb.tile([128, D], f32, tag="out")
                    nc.vector.scalar_tensor_tensor(
                        out=outt[:], in0=g01[:, 1, :], scalar=w1_all[:, i: i + 1],
                        in1=acc[:], op0=OP.mult, op1=OP.add,
                    )
                    nc.sync.dma_start(out=out_d[ts, :], in_=outt[:])

    if split_waits:
        _split_excess_waits(nc)
    return nc


def host_prep(x, norm_scale, w_router, w_up, w_down):
    """Shard x, fold norm_scale into router/up weights, build layouts."""
    x = np.asarray(x, dtype=np.float32)
    norm_scale = np.asarray(norm_scale, dtype=np.float32)
    w_router = np.asarray(w_router, dtype=np.float32)
    w_up = np.asarray(w_up, dtype=np.float32)
    w_down = np.asarray(w_down, dtype=np.float32)

    tokens = x.reshape(-1, D)
    shards = [
        np.ascontiguousarray(tokens[c * T_PER_CORE:(c + 1) * T_PER_CORE])
        for c in range(N_CORES)
    ]

    # router: [p, dc*8+e] = (w_router*ns).T[dc*128+p, e]
    wrT = (w_router * norm_scale[None, :]).T  # [D, E]
    wr = np.ascontiguousarray(
        wrT.reshape(8, 128, E).transpose(1, 0, 2).reshape(128, 8 * E)
    )

    # up: wuT[e, d, col] with cols = [u | g] -> [e, p, dp, kt, 2H] fp8 * SW
    wuT = (w_up * norm_scale[None, None, :]).transpose(0, 2, 1)  # [E, D, 2H]
    wu8 = np.ascontiguousarray(
        (wuT * SW).reshape(E, 4, 2, 128, 2 * H)
        .transpose(0, 3, 1, 2, 4)
        .reshape(E, 128, 8 * 2 * H)
    ).astype(ml_dtypes.float8_e4m3fn)

    # down: wdT[e, h, d]; [e, p, q, kt, d] = wdT[e, (q*2+kt)*128+p, d] fp8
    wdT = w_down.transpose(0, 2, 1)  # [E, H, D]
    wd8 = np.ascontiguousarray(
        wdT.reshape(E, 8, 2, 128, D).transpose(0, 3, 1, 2, 4)
        .reshape(E, 128, 8 * 2 * D)
    ).astype(ml_dtypes.float8_e4m3fn)

    ident8 = np.eye(128).astype(ml_dtypes.float8_e4m3fn)
    cum = np.triu(np.ones((128, 128)), k=1).astype(ml_dtypes.bfloat16)
    # tsel[(e',b'), (e,bb)] = (e'==e) & (b' < bb), bb in 0..8 (8 = totals)
    ep, bp = np.meshgrid(np.arange(E), np.arange(8), indexing="ij")
    epf = ep.reshape(-1)
    bpf = bp.reshape(-1)
    e2, bb = np.meshgrid(np.arange(E), np.arange(9), indexing="ij")
    e2f = e2.reshape(-1)
    bbf = bb.reshape(-1)
    tsel = ((epf[:, None] == e2f[None, :]) & (bpf[:, None] < bbf[None, :])).astype(
        np.float32
    )
    iota_ie = np.tile(np.arange(E, dtype=np.float32), (128, 4)).reshape(128, 32)
    iota_ei = np.tile(
        np.repeat(np.arange(E, dtype=np.float32), 4), (128, 1)
    ).reshape(128, 32)
    base8 = (np.arange(E, dtype=np.float32) * CCAP).reshape(1, E)

    common = {
        "wr": wr,
        "wu": wu8,
        "wd": wd8,
        "ident8": ident8,
        "cum": cum,
        "tsel": tsel,
        "iota_ie": iota_ie,
        "iota_ei": iota_ei,
        "base8": base8,
    }
    in_maps = []
    for c in range(N_CORES):
        sh = shards[c]
        xT = np.ascontiguousarray(
            sh.T.reshape(8, 128, T_PER_CORE).transpose(1, 0, 2)
            .reshape(128, 8 * T_PER_CORE)
        )
        in_maps.append(
            {
                "xbf": sh.astype(ml_dtypes.bfloat16),
                "xT": xT,
                **common,
            }
        )
    return in_maps


def _max_group_count(x, norm_scale, w_router):
    """Host-side routing replication to validate the capacity CEFF."""
    tokens = np.asarray(x, dtype=np.float32).reshape(-1, D)
    wrT = (np.asarray(w_router, dtype=np.float32)
           * np.asarray(norm_scale, dtype=np.float32)[None, :]).T
    mx = 0
    for c in range(N_CORES):
        sc = tokens[c * T_PER_CORE:(c + 1) * T_PER_CORE] @ wrT
        top1 = np.argmax(sc, axis=1)
        sc2 = sc.copy()
        sc2[np.arange(len(sc2)), top1] = -np.inf
        top2 = np.argmax(sc2, axis=1)
        cnts = np.bincount(top1, minlength=E) + np.bincount(top2, minlength=E)
        mx = max(mx, int(cnts.max()))
    return mx


def kernel(x, norm_scale, w_router, w_up, w_down):
    from concourse.bass_utils import run_bass_kernel_spmd

    ceff = CEFF
    mx = _max_group_count(x, norm_scale, w_router)
    if mx > CEFF - 4:
        ceff = CCAP  # fallback: full capacity (uneven inputs)
    key = ("nc", ceff)
    if key not in _CACHE:
        _CACHE[key] = build_program(ceff=ceff)
    nc = _CACHE[key]

    in_maps = host_prep(x, norm_scale, w_router, w_up, w_down)
    res = run_bass_kernel_spmd(nc, in_maps, core_ids=list(range(N_CORES)))
    out = np.concatenate([res.results[c]["out"] for c in range(N_CORES)], axis=0)
    return out.reshape(np.asarray(x).shape).astype(np.float32)


# revision 30
# speedup vs baseline: 1.1455x; 1.1455x over previous
"""MoE feed-forward (RMSNorm -> top-2 router -> SwiGLU experts -> combine)
on 8 TRN2 NeuronCores, data-parallel over tokens with all weights replicated.

v3: phase-A restructured (host-side x^T kills all router transposes+copies,
norm sum-of-squares on DVE, batched per-4-tile-group top-2 + grouping with
one prefix matmul, gates deferred to one batched sigmoid), Silu activation
(halves SwiGLU DVE work), effective capacity 576 (vs 640 storage; data max
568), fused scalar_tensor_tensor combine with bf16 skip kept in SBUF,
batched 2-offset combine gathers.

Per core (2048 tokens):
  - norm factor s/4 via DVE tensor_tensor_reduce; xn8 = x*(s/4) fp8
  - router scores f32 via PE with lhsT = x^T chunks (host layout)
  - per 4-tile group: batched top-2 + capacity-grouped permutation
    (rank matmul over 8 blocks, count matmul, prefix matmul vs Tsel)
  - xn8 rows scattered (indirect DMA) into per-expert groups in DRAM
  - per expert: gather fp8 rows, PE-transpose, fp8 DoubleRow up-GEMM
    (h^T layout), Silu+mult SwiGLU, fp8 DR down-GEMM, f16 writes
  - combine: batched gather both expert rows, (g0*w0+skip)+(g1*w1) STT

Self-contained: hardcodes all shapes; no file reads.
"""
import numpy as np
import ml_dtypes

T_PER_CORE = 2048
D = 1024
H = 2048
E = 8
N_CORES = 8
CCAP = 640   # per-(core,expert) storage capacity (row pitch)
CEFF = 576   # computed rows per group; actual seed-0 max count is 568
EPS = 1e-6
SW = 4.0     # weight scale folded into fp8 up-weights; acts carry 1/SW

NT = T_PER_CORE // 128   # 16 token tiles
NG = NT // 4             # 4 groups of 4 tiles
NB = 2 * NT              # 32 scatter blocks

_CACHE = {}


def _split_excess_waits(nc, max_waits=1):
    """walrus in this env caps sync-wait commands per instruction; move excess
    waits onto same-engine NOPs inserted just before the instruction."""
    import concourse.mybir as mybir

    n_split = 0
    for fn in nc.m.functions:
        for blk in fn.blocks:
            new_list = []
            for inst in blk.instructions:
                si = getattr(inst, "sync_info", None)
                waits = list(si.on_wait) if si is not None and si.on_wait else []
                if len(waits) > max_waits:
                    n_split += 1
                    excess = waits[: len(waits) - max_waits]
                    si.on_wait = waits[len(waits) - max_waits:]
                    for ci in range(0, len(excess), max_waits):
                        new_list.append(
                            mybir.InstNoOp(
                                name=f"waitsplit-{n_split}-{ci}",
                                engine=inst.engine,
                                ins=[],
                                outs=[],
                                sync_info=mybir.SyncInfo(
                                    on_wait=excess[ci: ci + max_waits], on_update=[]
                                ),
                            )
                        )
                new_list.append(inst)
            blk.instructions = new_list
    return n_split


def build_program(ceff=CEFF, split_waits=True, use_silu=True, batched_gather=True):
    import concourse.bass as bass
    import concourse.mybir as mybir
    import concourse.tile as tile

    f32 = mybir.dt.float32
    f16 = mybir.dt.float16
    bf16 = mybir.dt.bfloat16
    f8 = mybir.dt.float8e4
    i32 = mybir.dt.int32
    u8 = mybir.dt.uint8
    AF = mybir.ActivationFunctionType
    OP = mybir.AluOpType
    AX = mybir.AxisListType
    DR = mybir.MatmulPerfMode.DoubleRow

    CT = ceff - 512          # tail width (64)
    RT = (ceff + 127) // 128  # 5 gather/row tiles per expert (last partial)

    nc = bass.Bass()

    xbf_d = nc.declare_dram_parameter("xbf", [T_PER_CORE, D], bf16, isOutput=False)
    xT_d = nc.declare_dram_parameter("xT", [128, 8 * T_PER_CORE], f32, isOutput=False)
    wr_d = nc.declare_dram_parameter("wr", [128, 8 * E], f32, isOutput=False)
    wu_d = nc.declare_dram_parameter("wu", [E, 128, 8 * 2 * H], f8, isOutput=False)
    wd_d = nc.declare_dram_parameter("wd", [E, 128, 8 * 2 * D], f8, isOutput=False)
    ident8_d = nc.declare_dram_parameter("ident8", [128, 128], f8, isOutput=False)
    identf8_d = nc.declare_dram_parameter("identf8", [8, 8], f32, isOutput=False)
    cum_d = nc.declare_dram_parameter("cum", [128, 128], bf16, isOutput=False)
    tsel_d = nc.declare_dram_parameter("tsel", [64, 72], f32, isOutput=False)
    iota_ie_d = nc.declare_dram_parameter("iota_ie", [128, 32], f32, isOutput=False)
    iota_ei_d = nc.declare_dram_parameter("iota_ei", [128, 32], f32, isOutput=False)
    base8_d = nc.declare_dram_parameter("base8", [1, E], f32, isOutput=False)
    out_d = nc.declare_dram_parameter("out", [T_PER_CORE, D], f32, isOutput=True)

    g_dram = nc.dram_tensor("g_dram", [E * CCAP, D], f8)
    dn_dram = nc.dram_tensor("dn_dram", [E * CCAP, D], f16)

    with tile.TileContext(nc) as tc:
        with (
            tc.tile_pool(name="consts", bufs=1) as pc,
            tc.tile_pool(name="longl", bufs=1) as pl,
        ):
            ident8_sb = pc.tile_from(ident8_d[:])
            identf8_sb = pc.tile_from(identf8_d[:])
            cum_sb = pc.tile_from(cum_d[:])
            tsel_sb = pc.tile_from(tsel_d[:])
            iota_ie_sb = pc.tile_from(iota_ie_d[:])   # [128, 4, 8] views
            iota_ei_sb = pc.tile_from(iota_ei_d[:])   # [128, 8, 4] views
            wr_sb = pc.tile_from(wr_d[:])             # [128, 8, 8] views
            big_sb = pc.tile([128, 32], f32)
            nc.vector.memset(big_sb[:], 1e9)
            neg_sb = pc.tile([128, 32], f32)
            nc.vector.memset(neg_sb[:], -1e30)
            zeros64_sb = pc.tile([128, 64], f32)
            nc.vector.memset(zeros64_sb[:], 0.0)
            onesb1 = pc.tile([1, 128], f32)
            nc.vector.memset(onesb1[:], 1.0)
            ones128bf = pc.tile([128, 1], bf16)
            nc.vector.memset(ones128bf[:], 1.0)
            ones_col = pc.tile([128, 1], f32)
            nc.vector.memset(ones_col[:], 1.0)
            epsb_col = pc.tile([128, 1], f32)
            nc.vector.memset(epsb_col[:], EPS * SW * SW)

            s_all = pl.tile([128, NT], f32)
            ms_all = pl.tile([128, NT], f32)
            sd_all = pl.tile([128, NT], f32)
            gap_all = pl.tile([128, NT], f32)
            w0_all = pl.tile([128, NT], f32)
            w1_all = pl.tile([128, NT], f32)
            # dest_all layout: [p, tile*2 + k] (pairs adjacent for phase F)
            dest_all = pl.tile([128, NB], i32)
            xbf_all = pl.tile([128, NT, D], bf16)
            xn8_all = pl.tile([128, NT, D], f8)

            # ---------------- Phase A: norm, router, top-2, grouping, scatter
            with (
                tc.tile_pool(name="pxT", bufs=2) as pxT,
                tc.tile_pool(name="pscr", bufs=2) as pscr,
                tc.tile_pool(name="psmA", bufs=6) as psmA,
                tc.tile_pool(name="pog", bufs=2) as pog,
                tc.tile_pool(name="poffs", bufs=2) as poffs,
                tc.tile_pool(name="prun", bufs=2) as prun,
                tc.tile_pool(name="psc", bufs=2, space="PSUM") as psc,
                tc.tile_pool(name="ptrp", bufs=2, space="PSUM") as ptrp,
                tc.tile_pool(name="ppos", bufs=2, space="PSUM") as ppos,
                tc.tile_pool(name="pcnt", bufs=2, space="PSUM") as pcnt,
            ):
                run_row = prun.tile([1, E], f32, tag="run")
                nc.sync.dma_start(out=run_row[:], in_=base8_d[:])
                xT_view = xT_d[:].rearrange("p (dc t) -> p dc t", dc=8)
                for g in range(NG):
                    g4 = slice(4 * g, 4 * g + 4)
                    xT_g = pxT.tile([128, 8, 512], f32, tag="xT")
                    nc.sync.dma_start(
                        out=xT_g[:], in_=xT_view[:, :, g * 512:(g + 1) * 512]
                    )
                    for il in range(4):
                        i = 4 * g + il
                        ts = slice(i * 128, (i + 1) * 128)
                        nc.sync.dma_start(out=xbf_all[:, i, :], in_=xbf_d[ts, :])
                        scr = pscr.tile([128, D], bf16, tag="scr")
                        nc.vector.tensor_tensor(
                            scr[:], xbf_all[:, i, :], xbf_all[:, i, :], op=OP.mult
                        )
                        nc.vector.tensor_reduce(
                            ms_all[:, i: i + 1], scr[:], axis=AX.X, op=OP.add
                        )
                    # scores^T [e, 512 tok] via N=512 f32 matmuls, then
                    # transpose back to (tok, e) per tile
                    scpT = psc.tile([8, 512], f32, tag="scT")
                    for dc in range(8):
                        nc.tensor.matmul(
                            out=scpT[:],
                            lhsT=wr_sb[:].rearrange("p (dc e) -> p dc e", dc=8)[
                                :, dc, :
                            ],
                            rhs=xT_g[:, dc, :],
                            start=(dc == 0), stop=(dc == 7),
                        )
                    scT_sb = psmA.tile([8, 512], f32, tag="scTsb")
                    nc.vector.tensor_copy(scT_sb[:], scpT[:])
                    s_sb = psmA.tile([128, 4, E], f32, tag="ssb")
                    for il in range(4):
                        trp = ptrp.tile([128, E], f32, tag="trp")
                        nc.tensor.transpose(
                            trp[:], scT_sb[:, il * 128:(il + 1) * 128],
                            identf8_sb[:],
                        )
                        nc.vector.tensor_copy(s_sb[:, il, :], trp[:])

                    # norm factors for the group
                    nc.scalar.activation(
                        sd_all[:, g4], ms_all[:, g4], AF.Sqrt,
                        bias=epsb_col[:], scale=SW * SW / D,
                    )
                    nc.vector.reciprocal(s_all[:, g4], sd_all[:, g4])
                    for il in range(4):
                        i = 4 * g + il
                        nc.vector.tensor_scalar_mul(
                            xn8_all[:, i, :], xbf_all[:, i, :], s_all[:, i: i + 1]
                        )

                    # batched top-2 on [128, 4, 8]
                    iota_ie = iota_ie_sb[:].rearrange("p (i e) -> p i e", i=4)
                    iota_ei = iota_ei_sb[:].rearrange("p (e i) -> p e i", e=8)
                    big_ie = big_sb[:].rearrange("p (i e) -> p i e", i=4)
                    neg_ie = neg_sb[:].rearrange("p (i e) -> p i e", i=4)
                    m0 = psmA.tile([128, 4], f32, tag="m0")
                    nc.vector.tensor_reduce(m0[:], s_sb[:], axis=AX.X, op=OP.max)
                    eq0 = psmA.tile([128, 4, E], u8, tag="eq")
                    nc.vector.tensor_tensor(
                        eq0[:], s_sb[:], m0[:].unsqueeze(2).to_broadcast([128, 4, E]),
                        op=OP.is_equal,
                    )
                    cand = psmA.tile([128, 4, E], f32, tag="cand")
                    nc.vector.select(cand[:], eq0[:], iota_ie, big_ie)
                    i0f = psmA.tile([128, 4], f32, tag="i0")
                    nc.vector.tensor_reduce(i0f[:], cand[:], axis=AX.X, op=OP.min)
                    eqA = psmA.tile([128, 4, E], u8, tag="eq")
                    nc.vector.tensor_tensor(
                        eqA[:], iota_ie, i0f[:].unsqueeze(2).to_broadcast([128, 4, E]),
                        op=OP.is_equal,
                    )
                    sc2 = psmA.tile([128, 4, E], f32, tag="sc2")
                    nc.vector.select(sc2[:], eqA[:], neg_ie, s_sb[:])
                    m1 = psmA.tile([128, 4], f32, tag="m1")
                    nc.vector.tensor_reduce(m1[:], sc2[:], axis=AX.X, op=OP.max)
                    eq1 = psmA.tile([128, 4, E], u8, tag="eq")
                    nc.vector.tensor_tensor(
                        eq1[:], sc2[:], m1[:].unsqueeze(2).to_broadcast([128, 4, E]),
                        op=OP.is_equal,
                    )
                    cand1 = psmA.tile([128, 4, E], f32, tag="cand")
                    nc.vector.select(cand1[:], eq1[:], iota_ie, big_ie)
                    i1f = psmA.tile([128, 4], f32, tag="i1")
                    nc.vector.tensor_reduce(i1f[:], cand1[:], axis=AX.X, op=OP.min)

                    # gates (sigmoid deferred): gap = (m0-m1)*s
                    nc.vector.tensor_sub(gap_all[:, g4], m0[:], m1[:])
                    nc.vector.tensor_tensor(
                        gap_all[:, g4], gap_all[:, g4], s_all[:, g4], op=OP.mult
                    )

                    # one-hots in (e, block) layout; block b = k*4 + il
                    ohu_g = pog.tile([128, E, 8], u8, tag="ohu")
                    nc.vector.tensor_tensor(
                        ohu_g[:, :, 0:4], iota_ei,
                        i0f[:].unsqueeze(1).to_broadcast([128, E, 4]),
                        op=OP.is_equal,
                    )
                    nc.vector.tensor_tensor(
                        ohu_g[:, :, 4:8], iota_ei,
                        i1f[:].unsqueeze(1).to_broadcast([128, E, 4]),
                        op=OP.is_equal,
                    )
                    oh_g = pog.tile([128, E, 8], bf16, tag="oh")
                    nc.vector.tensor_copy(oh_g[:], ohu_g[:])

                    # counts per (e,b): [64,1] = oh^T @ ones
                    oh_flat = oh_g[:].rearrange("p e b -> p (e b)")
                    cntp = pcnt.tile([64, 1], f32, tag="cntoff")
                    nc.tensor.matmul(
                        out=cntp[:], lhsT=oh_flat, rhs=ones128bf[:],
                        start=True, stop=True,
                    )
                    cnt_sb = psmA.tile([64, 1], f32, tag="cntsb")
                    nc.vector.tensor_copy(cnt_sb[:], cntp[:])
                    # intra-group exclusive prefix over blocks (col 8: totals)
                    offp = pcnt.tile([1, 72], f32, tag="cntoff")
                    nc.tensor.matmul(
                        out=offp[:], lhsT=cnt_sb[:], rhs=tsel_sb[:],
                        start=True, stop=True,
                    )
                    offs_sb = poffs.tile([1, E, 9], f32, tag="offs")
                    nc.vector.tensor_tensor(
                        offs_sb[:], offp[:].rearrange("p (e b) -> p e b", e=8),
                        run_row[:].unsqueeze(2).to_broadcast([1, E, 9]),
                        op=OP.add,
                    )
                    run_next = prun.tile([1, E], f32, tag="run")
                    nc.vector.tensor_copy(
                        run_next[:].unsqueeze(2), offs_sb[:, :, 8:9]
                    )
                    run_row = run_next

                    # dest = rank-in-block + group offset, per (token, e, b)
                    pos = ppos.tile([128, E, 8], f32, tag="pos")
                    nc.tensor.matmul(
                        out=pos[:].rearrange("p e b -> p (e b)"), lhsT=cum_sb[:],
                        rhs=oh_flat, start=True, stop=False, skip_group_check=True,
                    )
                    nc.tensor.matmul(
                        out=pos[:].rearrange("p e b -> p (e b)"), lhsT=onesb1[:],
                        rhs=offs_sb[:, :, 0:8], start=False, stop=True,
                        skip_group_check=True,
                    )
                    seld = psmA.tile([128, E, 8], f32, tag="seld")
                    nc.vector.select(
                        seld[:], ohu_g[:], pos[:],
                        zeros64_sb[:].rearrange("p (e b) -> p e b", e=8),
                    )
                    destf = psmA.tile([128, 8], f32, tag="destf")
                    nc.vector.tensor_reduce(
                        destf[:], seld[:].transpose([0, 2, 1]), axis=AX.X, op=OP.add
                    )
                    dest_tk = dest_all[:].rearrange("p (t k) -> p t k", k=2)
                    nc.vector.tensor_copy(
                        dest_tk[:, 4 * g: 4 * g + 4, 0:1],
                        destf[:, 0:4].unsqueeze(2),
                    )
                    nc.vector.tensor_copy(
                        dest_tk[:, 4 * g: 4 * g + 4, 1:2],
                        destf[:, 4:8].unsqueeze(2),
                    )
                    for b in range(8):
                        col = (4 * g + (b % 4)) * 2 + (b // 4)
                        nc.gpsimd.indirect_dma_start(
                            out=g_dram[:],
                            out_offset=bass.IndirectOffsetOnAxis(
                                ap=dest_all[:, col: col + 1], axis=0
                            ),
                            in_=xn8_all[:, 4 * g + (b % 4), :],
                            in_offset=None,
                        )

                # batched gates
                nc.scalar.activation(w0_all[:], gap_all[:], AF.Sigmoid, scale=SW)
                nc.vector.tensor_tensor(
                    w1_all[:], ones_col[:].to_broadcast([128, NT]), w0_all[:],
                    op=OP.subtract,
                )

            # ---------------- Phase E: expert FFN loop
            with (
                tc.tile_pool(name="pgr", bufs=3) as pgr,
                tc.tile_pool(name="pgts", bufs=2) as pgts,
                tc.tile_pool(name="pwu", bufs=2) as pwu,
                tc.tile_pool(name="pwd", bufs=2) as pwd,
                tc.tile_pool(name="pht", bufs=1) as pht,
                tc.tile_pool(name="psil", bufs=2) as psil,
                tc.tile_pool(name="psilt", bufs=2) as psilt,
                tc.tile_pool(name="pdo", bufs=3) as pdo,
                tc.tile_pool(name="ppmain", bufs=2, space="PSUM") as ppmain,
                tc.tile_pool(name="pptail", bufs=2, space="PSUM") as pptail,
                tc.tile_pool(name="ppd", bufs=2, space="PSUM") as ppd,
            ):
                for e in range(E):
                    wu_sb = pwu.tile([128, 4, 2, 2 * H], f8, tag="wu")
                    nc.sync.dma_start(
                        out=wu_sb[:].rearrange("p dp kt c -> p (dp kt c)"),
                        in_=wu_d[e],
                    )
                    wd_sb = pwd.tile([128, 8, 2, D], f8, tag="wd")
                    nc.sync.dma_start(
                        out=wd_sb[:].rearrange("p q kt d -> p (q kt d)"), in_=wd_d[e]
                    )

                    # gather fp8 rows + transpose to [d, rows]
                    gts = pgts.tile([128, 8, ceff], f8, tag="gts")
                    for rt in range(RT):
                        w = min(128, ceff - rt * 128)
                        gr = pgr.tile([128, D], f8, tag="gr")
                        nc.sync.dma_start(
                            out=gr[:],
                            in_=g_dram[
                                e * CCAP + rt * 128: e * CCAP + (rt + 1) * 128, :
                            ],
                        )
                        for half in range(2):
                            # fp8 transpose hw mode writes with element step 2
                            trb = ppmain.tile([128, 4, 128, 2], f8, tag="pug")
                            for q in range(4):
                                dc = half * 4 + q
                                nc.tensor.transpose(
                                    trb[:, q, :, 0],
                                    gr[:, dc * 128:(dc + 1) * 128],
                                    ident8_sb[:],
                                )
                            nc.vector.tensor_copy(
                                gts[:, half * 4:(half + 1) * 4,
                                    rt * 128: rt * 128 + w],
                                trb[:, :, 0:w, 0],
                            )

                    # up-GEMM (fp8 DoubleRow, h^T layout) + SwiGLU via Silu
                    hts = pht.tile([128, 16, ceff], f8, tag="ht")
                    for hg in range(16):
                        pug = ppmain.tile([128, 1024], f32, tag="pug")
                        pugt = pptail.tile([128, 2 * CT], f32, tag="pugt")
                        for part in range(2):
                            col0 = hg * 128 + part * H
                            for dp in range(4):
                                nc.tensor.matmul(
                                    out=pug[:, part * 512:(part + 1) * 512],
                                    lhsT=wu_sb[:, dp, :, col0:col0 + 128],
                                    rhs=gts[:, 2 * dp:2 * dp + 2, 0:512],
                                    start=(dp == 0), stop=(dp == 3), perf_mode=DR,
                                )
                            for dp in range(4):
                                nc.tensor.matmul(
                                    out=pugt[:, part * CT:(part + 1) * CT],
                                    lhsT=wu_sb[:, dp, :, col0:col0 + 128],
                                    rhs=gts[:, 2 * dp:2 * dp + 2, 512:ceff],
                                    start=(dp == 0), stop=(dp == 3), perf_mode=DR,
                                )
                        for (pt, po, cs, w, pp) in (
                            (pug, 512, slice(0, 512), 512, psil),
                            (pugt, CT, slice(512, ceff), CT, psilt),
                        ):
                            sil = pp.tile([128, w], f32, tag=f"sil{w}")
                            if use_silu:
                                nc.scalar.activation(
                                    sil[:], pt[:, po:2 * po], AF.Silu
                                )
                                nc.vector.tensor_tensor(
                                    hts[:, hg, cs], pt[:, 0:po], sil[:],
                                    op=OP.mult,
                                )
                            else:
                                nc.scalar.activation(
                                    sil[:], pt[:, po:2 * po], AF.Sigmoid
                                )
                                h1 = pp.tile([128, w], f32, tag=f"h1{w}")
                                nc.vector.tensor_tensor(
                                    h1[:], pt[:, 0:po], sil[:], op=OP.mult
                                )
                                nc.vector.tensor_tensor(
                                    hts[:, hg, cs], h1[:], pt[:, po:2 * po],
                                    op=OP.mult,
                                )

                    # down-GEMM (fp8 DR)
                    for rt in range(RT):
                        m = min(128, ceff - rt * 128)
                        rs = slice(rt * 128, rt * 128 + m)
                        do = pdo.tile([128, D], f16, tag="do")
                        for dq in range(2):
                            pd = ppd.tile([128, 512], f32, tag="pd")
                            for q in range(8):
                                nc.tensor.matmul(
                                    out=pd[0:m, :],
                                    lhsT=hts[:, 2 * q:2 * q + 2, rs],
                                    rhs=wd_sb[:, q, :, dq * 512:(dq + 1) * 512],
                                    start=(q == 0), stop=(q == 7), perf_mode=DR,
                                )
                            nc.scalar.copy(
                                do[0:m, dq * 512:(dq + 1) * 512], pd[0:m, :]
                            )
                        nc.sync.dma_start(
                            out=dn_dram[e * CCAP + rt * 128: e * CCAP + rt * 128 + m, :],
                            in_=do[0:m, :],
                        )

            # ---------------- Phase F: combine
            with (
                tc.tile_pool(name="pgd", bufs=6) as pgd,
                tc.tile_pool(name="pcmb", bufs=3) as pcmb,
            ):
                for i in range(NT):
                    ts = slice(i * 128, (i + 1) * 128)
                    g01 = pgd.tile([128, 2, D], f16, tag="gd")
                    if batched_gather:
                        nc.gpsimd.indirect_dma_start(
                            out=g01[:],
                            out_offset=None,
                            in_=dn_dram[:],
                            in_offset=bass.IndirectOffsetOnAxis(
                                ap=dest_all[:, 2 * i: 2 * i + 2], axis=0
                            ),
                        )
                    else:
                        for k in range(2):
                            nc.gpsimd.indirect_dma_start(
                                out=g01[:, k, :],
                                out_offset=None,
                                in_=dn_dram[:],
                                in_offset=bass.IndirectOffsetOnAxis(
                                    ap=dest_all[:, 2 * i + k: 2 * i + k + 1],
                                    axis=0,
                                ),
                            )
                    acc = pcmb.tile([128, D], f32, tag="acc")
                    nc.vector.scalar_tensor_tensor(
                        out=acc[:], in0=g01[:, 0, :], scalar=w0_all[:, i: i + 1],
                        in1=xbf_all[:, i, :], op0=OP.mult, op1=OP.add,
                    )
                    outt = pcmb.tile([128, D], f32, tag="out")
                    nc.vector.scalar_tensor_tensor(
                        out=outt[:], in0=g01[:, 1, :], scalar=w1_all[:, i: i + 1],
                        in1=acc[:], op0=OP.mult, op1=OP.add,
                    )
                    nc.sync.dma_start(out=out_d[ts, :], in_=outt[:])

    if split_waits:
        _split_excess_waits(nc)
    return nc


def host_prep(x, norm_scale, w_router, w_up, w_down):
    """Shard x, fold norm_scale into router/up weights, build layouts."""
    x = np.asarray(x, dtype=np.float32)
    norm_scale = np.asarray(norm_scale, dtype=np.float32)
    w_router = np.asarray(w_router, dtype=np.float32)
    w_up = np.asarray(w_up, dtype=np.float32)
    w_down = np.asarray(w_down, dtype=np.float32)

    tokens = x.reshape(-1, D)
    shards = [
        np.ascontiguousarray(tokens[c * T_PER_CORE:(c + 1) * T_PER_CORE])
        for c in range(N_CORES)
    ]

    # router: [p, dc*8+e] = (w_router*ns).T[dc*128+p, e]
    wrT = (w_router * norm_scale[None, :]).T  # [D, E]
    wr = np.ascontiguousarray(
        wrT.reshape(8, 128, E).transpose(1, 0, 2).reshape(128, 8 * E)
    )

    # up: wuT[e, d, col] with cols = [u | g] -> [e, p, dp, kt, 2H] fp8 * SW
    wuT = (w_up * norm_scale[None, None, :]).transpose(0, 2, 1)  # [E, D, 2H]
    wu8 = np.ascontiguousarray(
        (wuT * SW).reshape(E, 4, 2, 128, 2 * H)
        .transpose(0, 3, 1, 2, 4)
        .reshape(E, 128, 8 * 2 * H)
    ).astype(ml_dtypes.float8_e4m3fn)

    # down: wdT[e, h, d]; [e, p, q, kt, d] = wdT[e, (q*2+kt)*128+p, d] fp8
    wdT = w_down.transpose(0, 2, 1)  # [E, H, D]
    wd8 = np.ascontiguousarray(
        wdT.reshape(E, 8, 2, 128, D).transpose(0, 3, 1, 2, 4)
        .reshape(E, 128, 8 * 2 * D)
    ).astype(ml_dtypes.float8_e4m3fn)

    ident8 = np.eye(128).astype(ml_dtypes.float8_e4m3fn)
    identf8 = np.eye(8, dtype=np.float32)
    cum = np.triu(np.ones((128, 128)), k=1).astype(ml_dtypes.bfloat16)
    # tsel[(e',b'), (e,bb)] = (e'==e) & (b' < bb), bb in 0..8 (8 = totals)
    ep, bp = np.meshgrid(np.arange(E), np.arange(8), indexing="ij")
    epf = ep.reshape(-1)
    bpf = bp.reshape(-1)
    e2, bb = np.meshgrid(np.arange(E), np.arange(9), indexing="ij")
    e2f = e2.reshape(-1)
    bbf = bb.reshape(-1)
    tsel = ((epf[:, None] == e2f[None, :]) & (bpf[:, None] < bbf[None, :])).astype(
        np.float32
    )
    iota_ie = np.tile(np.arange(E, dtype=np.float32), (128, 4)).reshape(128, 32)
    iota_ei = np.tile(
        np.repeat(np.arange(E, dtype=np.float32), 4), (128, 1)
    ).reshape(128, 32)
    base8 = (np.arange(E, dtype=np.float32) * CCAP).reshape(1, E)

    common = {
        "wr": wr,
        "wu": wu8,
        "wd": wd8,
        "ident8": ident8,
        "identf8": identf8,
        "cum": cum,
        "tsel": tsel,
        "iota_ie": iota_ie,
        "iota_ei": iota_ei,
        "base8": base8,
    }
    in_maps = []
    for c in range(N_CORES):
        sh = shards[c]
        xT = np.ascontiguousarray(
            sh.T.reshape(8, 128, T_PER_CORE).transpose(1, 0, 2)
            .reshape(128, 8 * T_PER_CORE)
        )
        in_maps.append(
            {
                "xbf": sh.astype(ml_dtypes.bfloat16),
                "xT": xT,
                **common,
            }
        )
    return in_maps


def _max_group_count(x, norm_scale, w_router):
    """Host-side routing replication to validate the capacity CEFF."""
    tokens = np.asarray(x, dtype=np.float32).reshape(-1, D)
    wrT = (np.asarray(w_router, dtype=np.float32)
           * np.asarray(norm_scale, dtype=np.float32)[None, :]).T
    mx = 0
    for c in range(N_CORES):
        sc = tokens[c * T_PER_CORE:(c + 1) * T_PER_CORE] @ wrT
        top1 = np.argmax(sc, axis=1)
        sc2 = sc.copy()
        sc2[np.arange(len(sc2)), top1] = -np.inf
        top2 = np.argmax(sc2, axis=1)
        cnts = np.bincount(top1, minlength=E) + np.bincount(top2, minlength=E)
        mx = max(mx, int(cnts.max()))
    return mx


def kernel(x, norm_scale, w_router, w_up, w_down):
    from concourse.bass_utils import run_bass_kernel_spmd

    ceff = CEFF
    mx = _max_group_count(x, norm_scale, w_router)
    if mx > CEFF - 4:
        ceff = CCAP  # fallback: full capacity (uneven inputs)
    key = ("nc", ceff)
    if key not in _CACHE:
        _CACHE[key] = build_program(ceff=ceff, batched_gather=False)
    nc = _CACHE[key]

    in_maps = host_prep(x, norm_scale, w_router, w_up, w_down)
    res = run_bass_kernel_spmd(nc, in_maps, core_ids=list(range(N_CORES)))
    out = np.concatenate([res.results[c]["out"] for c in range(N_CORES)], axis=0)
    return out.reshape(np.asarray(x).shape).astype(np.float32)


# revision 32
# speedup vs baseline: 1.2592x; 1.0993x over previous
"""MoE feed-forward (RMSNorm -> top-2 router -> SwiGLU experts -> combine)
on 8 TRN2 NeuronCores, data-parallel over tokens with all weights replicated.

v3: phase-A restructured (host-side x^T kills all router transposes+copies,
norm sum-of-squares on DVE, batched per-4-tile-group top-2 + grouping with
one prefix matmul, gates deferred to one batched sigmoid), Silu activation
(halves SwiGLU DVE work), effective capacity 576 (vs 640 storage; data max
568), fused scalar_tensor_tensor combine with bf16 skip kept in SBUF,
batched 2-offset combine gathers.

Per core (2048 tokens):
  - norm factor s/4 via DVE tensor_tensor_reduce; xn8 = x*(s/4) fp8
  - router scores f32 via PE with lhsT = x^T chunks (host layout)
  - per 4-tile group: batched top-2 + capacity-grouped permutation
    (rank matmul over 8 blocks, count matmul, prefix matmul vs Tsel)
  - xn8 rows scattered (indirect DMA) into per-expert groups in DRAM
  - per expert: gather fp8 rows, PE-transpose, fp8 DoubleRow up-GEMM
    (h^T layout), Silu+mult SwiGLU, fp8 DR down-GEMM, f16 writes
  - combine: batched gather both expert rows, (g0*w0+skip)+(g1*w1) STT

Self-contained: hardcodes all shapes; no file reads.
"""
import numpy as np
import ml_dtypes

T_PER_CORE = 2048
D = 1024
H = 2048
E = 8
N_CORES = 8
CCAP = 640   # per-(core,expert) storage capacity (row pitch)
CEFF = 576   # computed rows per group; actual seed-0 max count is 568
EPS = 1e-6
SW = 4.0     # weight scale folded into fp8 up-weights; acts carry 1/SW

NT = T_PER_CORE // 128   # 16 token tiles
NG = NT // 4             # 4 groups of 4 tiles
NB = 2 * NT              # 32 scatter blocks

_CACHE = {}


def _split_excess_waits(nc, max_waits=1):
    """walrus in this env caps sync-wait commands per instruction; move excess
    waits onto same-engine NOPs inserted just before the instruction."""
    import concourse.mybir as mybir

    n_split = 0
    for fn in nc.m.functions:
        for blk in fn.blocks:
            new_list = []
            for inst in blk.instructions:
                si = getattr(inst, "sync_info", None)
                waits = list(si.on_wait) if si is not None and si.on_wait else []
                if len(waits) > max_waits:
                    n_split += 1
                    excess = waits[: len(waits) - max_waits]
                    si.on_wait = waits[len(waits) - max_waits:]
                    for ci in range(0, len(excess), max_waits):
                        new_list.append(
                            mybir.InstNoOp(
                                name=f"waitsplit-{n_split}-{ci}",
                                engine=inst.engine,
                                ins=[],
                                outs=[],
                                sync_info=mybir.SyncInfo(
                                    on_wait=excess[ci: ci + max_waits], on_update=[]
                                ),
                            )
                        )
                new_list.append(inst)
            blk.instructions = new_list
    return n_split


def _strip_scatter_waw(nc):
    """The 32 phase-A indirect scatters write provably-disjoint g_dram rows
    (capacity grouping assigns unique slots); the tile framework still chains
    them with write-after-write completion waits, serializing ~3.5us each.
    Drop waits on semaphores that are only updated by sibling scatters."""
    scatters = []
    for fn in nc.m.functions:
        for blk in fn.blocks:
            for inst in blk.instructions:
                for o in (getattr(inst, "outs", []) or []):
                    if "g_dram" in str(getattr(o, "memref", "")):
                        scatters.append(inst)
                        break
    upd_ids = {
        u.id
        for s in scatters
        if s.sync_info is not None
        for u in (s.sync_info.on_update or [])
    }
    n = 0
    for s in scatters:
        si = s.sync_info
        if si is None or not si.on_wait:
            continue
        kept = [w for w in si.on_wait if w.id not in upd_ids]
        n += len(si.on_wait) - len(kept)
        si.on_wait = kept
    return n


def build_program(ceff=CEFF, split_waits=True, use_silu=True, batched_gather=True,
                  strip_waw=True):
    import concourse.bass as bass
    import concourse.mybir as mybir
    import concourse.tile as tile

    f32 = mybir.dt.float32
    f16 = mybir.dt.float16
    bf16 = mybir.dt.bfloat16
    f8 = mybir.dt.float8e4
    i32 = mybir.dt.int32
    u8 = mybir.dt.uint8
    AF = mybir.ActivationFunctionType
    OP = mybir.AluOpType
    AX = mybir.AxisListType
    DR = mybir.MatmulPerfMode.DoubleRow

    CT = ceff - 512          # tail width (64)
    RT = (ceff + 127) // 128  # 5 gather/row tiles per expert (last partial)

    nc = bass.Bass()

    xbf_d = nc.declare_dram_parameter("xbf", [T_PER_CORE, D], bf16, isOutput=False)
    xT_d = nc.declare_dram_parameter("xT", [128, 8 * T_PER_CORE], f32, isOutput=False)
    wr_d = nc.declare_dram_parameter("wr", [128, 8 * E], f32, isOutput=False)
    wu_d = nc.declare_dram_parameter("wu", [E, 128, 8 * 2 * H], f8, isOutput=False)
    wd_d = nc.declare_dram_parameter("wd", [E, 128, 8 * 2 * D], f8, isOutput=False)
    ident8_d = nc.declare_dram_parameter("ident8", [128, 128], f8, isOutput=False)
    identf8_d = nc.declare_dram_parameter("identf8", [8, 8], f32, isOutput=False)
    cum_d = nc.declare_dram_parameter("cum", [128, 128], bf16, isOutput=False)
    tsel_d = nc.declare_dram_parameter("tsel", [64, 72], f32, isOutput=False)
    iota_ie_d = nc.declare_dram_parameter("iota_ie", [128, 32], f32, isOutput=False)
    iota_ei_d = nc.declare_dram_parameter("iota_ei", [128, 32], f32, isOutput=False)
    base8_d = nc.declare_dram_parameter("base8", [1, E], f32, isOutput=False)
    out_d = nc.declare_dram_parameter("out", [T_PER_CORE, D], f32, isOutput=True)

    g_dram = nc.dram_tensor("g_dram", [E * CCAP, D], f8)
    dn_dram = nc.dram_tensor("dn_dram", [E * CCAP, D], f16)

    with tile.TileContext(nc) as tc:
        with (
            tc.tile_pool(name="consts", bufs=1) as pc,
            tc.tile_pool(name="longl", bufs=1) as pl,
        ):
            ident8_sb = pc.tile_from(ident8_d[:])
            identf8_sb = pc.tile_from(identf8_d[:])
            cum_sb = pc.tile_from(cum_d[:])
            tsel_sb = pc.tile_from(tsel_d[:])
            iota_ie_sb = pc.tile_from(iota_ie_d[:])   # [128, 4, 8] views
            iota_ei_sb = pc.tile_from(iota_ei_d[:])   # [128, 8, 4] views
            wr_sb = pc.tile_from(wr_d[:])             # [128, 8, 8] views
            big_sb = pc.tile([128, 32], f32)
            nc.vector.memset(big_sb[:], 1e9)
            neg_sb = pc.tile([128, 32], f32)
            nc.vector.memset(neg_sb[:], -1e30)
            zeros64_sb = pc.tile([128, 64], f32)
            nc.vector.memset(zeros64_sb[:], 0.0)
            onesb1 = pc.tile([1, 128], f32)
            nc.vector.memset(onesb1[:], 1.0)
            ones128bf = pc.tile([128, 1], bf16)
            nc.vector.memset(ones128bf[:], 1.0)
            ones_col = pc.tile([128, 1], f32)
            nc.vector.memset(ones_col[:], 1.0)
            epsb_col = pc.tile([128, 1], f32)
            nc.vector.memset(epsb_col[:], EPS * SW * SW)

            s_all = pl.tile([128, NT], f32)
            ms_all = pl.tile([128, NT], f32)
            sd_all = pl.tile([128, NT], f32)
            gap_all = pl.tile([128, NT], f32)
            w0_all = pl.tile([128, NT], f32)
            w1_all = pl.tile([128, NT], f32)
            # dest_all layout: [p, tile*2 + k] (pairs adjacent for phase F)
            dest_all = pl.tile([128, NB], i32)
            xbf_all = pl.tile([128, NT, D], bf16)
            xn8_all = pl.tile([128, NT, D], f8)

            # ---------------- Phase A: norm, router, top-2, grouping, scatter
            with (
                tc.tile_pool(name="pxT", bufs=2) as pxT,
                tc.tile_pool(name="pscr", bufs=2) as pscr,
                tc.tile_pool(name="psmA", bufs=6) as psmA,
                tc.tile_pool(name="pog", bufs=2) as pog,
                tc.tile_pool(name="poffs", bufs=2) as poffs,
                tc.tile_pool(name="prun", bufs=2) as prun,
                tc.tile_pool(name="psc", bufs=2, space="PSUM") as psc,
                tc.tile_pool(name="ptrp", bufs=2, space="PSUM") as ptrp,
                tc.tile_pool(name="ppos", bufs=2, space="PSUM") as ppos,
                tc.tile_pool(name="pcnt", bufs=2, space="PSUM") as pcnt,
            ):
                run_row = prun.tile([1, E], f32, tag="run")
                nc.sync.dma_start(out=run_row[:], in_=base8_d[:])
                xT_view = xT_d[:].rearrange("p (dc t) -> p dc t", dc=8)
                for g in range(NG):
                    g4 = slice(4 * g, 4 * g + 4)
                    xT_g = pxT.tile([128, 8, 512], f32, tag="xT")
                    nc.sync.dma_start(
                        out=xT_g[:], in_=xT_view[:, :, g * 512:(g + 1) * 512]
                    )
                    for il in range(4):
                        i = 4 * g + il
                        ts = slice(i * 128, (i + 1) * 128)
                        nc.sync.dma_start(out=xbf_all[:, i, :], in_=xbf_d[ts, :])
                        scr = pscr.tile([128, D], bf16, tag="scr")
                        nc.vector.tensor_tensor(
                            scr[:], xbf_all[:, i, :], xbf_all[:, i, :], op=OP.mult
                        )
                        nc.vector.tensor_reduce(
                            ms_all[:, i: i + 1], scr[:], axis=AX.X, op=OP.add
                        )
                    # scores^T [e, 512 tok] via N=512 f32 matmuls, then
                    # transpose back to (tok, e) per tile
                    scpT = psc.tile([8, 512], f32, tag="scT")
                    for dc in range(8):
                        nc.tensor.matmul(
                            out=scpT[:],
                            lhsT=wr_sb[:].rearrange("p (dc e) -> p dc e", dc=8)[
                                :, dc, :
                            ],
                            rhs=xT_g[:, dc, :],
                            start=(dc == 0), stop=(dc == 7),
                        )
                    scT_sb = psmA.tile([8, 512], f32, tag="scTsb")
                    nc.vector.tensor_copy(scT_sb[:], scpT[:])
                    s_sb = psmA.tile([128, 4, E], f32, tag="ssb")
                    for il in range(4):
                        trp = ptrp.tile([128, E], f32, tag="trp")
                        nc.tensor.transpose(
                            trp[:], scT_sb[:, il * 128:(il + 1) * 128],
                            identf8_sb[:],
                        )
                        nc.vector.tensor_copy(s_sb[:, il, :], trp[:])

                    # norm factors for the group
                    nc.scalar.activation(
                        sd_all[:, g4], ms_all[:, g4], AF.Sqrt,
                        bias=epsb_col[:], scale=SW * SW / D,
                    )
                    nc.vector.reciprocal(s_all[:, g4], sd_all[:, g4])
                    for il in range(4):
                        i = 4 * g + il
                        nc.vector.tensor_scalar_mul(
                            xn8_all[:, i, :], xbf_all[:, i, :], s_all[:, i: i + 1]
                        )

                    # batched top-2 on [128, 4, 8]
                    iota_ie = iota_ie_sb[:].rearrange("p (i e) -> p i e", i=4)
                    iota_ei = iota_ei_sb[:].rearrange("p (e i) -> p e i", e=8)
                    big_ie = big_sb[:].rearrange("p (i e) -> p i e", i=4)
                    neg_ie = neg_sb[:].rearrange("p (i e) -> p i e", i=4)
                    m0 = psmA.tile([128, 4], f32, tag="m0")
                    nc.vector.tensor_reduce(m0[:], s_sb[:], axis=AX.X, op=OP.max)
                    eq0 = psmA.tile([128, 4, E], u8, tag="eq")
                    nc.vector.tensor_tensor(
                        eq0[:], s_sb[:], m0[:].unsqueeze(2).to_broadcast([128, 4, E]),
                        op=OP.is_equal,
                    )
                    cand = psmA.tile([128, 4, E], f32, tag="cand")
                    nc.vector.select(cand[:], eq0[:], iota_ie, big_ie)
                    i0f = psmA.tile([128, 4], f32, tag="i0")
                    nc.vector.tensor_reduce(i0f[:], cand[:], axis=AX.X, op=OP.min)
                    eqA = psmA.tile([128, 4, E], u8, tag="eq")
                    nc.vector.tensor_tensor(
                        eqA[:], iota_ie, i0f[:].unsqueeze(2).to_broadcast([128, 4, E]),
                        op=OP.is_equal,
                    )
                    sc2 = psmA.tile([128, 4, E], f32, tag="sc2")
                    nc.vector.select(sc2[:], eqA[:], neg_ie, s_sb[:])
                    m1 = psmA.tile([128, 4], f32, tag="m1")
                    nc.vector.tensor_reduce(m1[:], sc2[:], axis=AX.X, op=OP.max)
                    eq1 = psmA.tile([128, 4, E], u8, tag="eq")
                    nc.vector.tensor_tensor(
                        eq1[:], sc2[:], m1[:].unsqueeze(2).to_broadcast([128, 4, E]),
                        op=OP.is_equal,
                    )
                    cand1 = psmA.tile([128, 4, E], f32, tag="cand")
                    nc.vector.select(cand1[:], eq1[:], iota_ie, big_ie)
                    i1f = psmA.tile([128, 4], f32, tag="i1")
                    nc.vector.tensor_reduce(i1f[:], cand1[:], axis=AX.X, op=OP.min)

                    # gates (sigmoid deferred): gap = (m0-m1)*s
                    nc.vector.tensor_sub(gap_all[:, g4], m0[:], m1[:])
                    nc.vector.tensor_tensor(
                        gap_all[:, g4], gap_all[:, g4], s_all[:, g4], op=OP.mult
                    )

                    # one-hots in (e, block) layout; block b = k*4 + il
                    ohu_g = pog.tile([128, E, 8], u8, tag="ohu")
                    nc.vector.tensor_tensor(
                        ohu_g[:, :, 0:4], iota_ei,
                        i0f[:].unsqueeze(1).to_broadcast([128, E, 4]),
                        op=OP.is_equal,
                    )
                    nc.vector.tensor_tensor(
                        ohu_g[:, :, 4:8], iota_ei,
                        i1f[:].unsqueeze(1).to_broadcast([128, E, 4]),
                        op=OP.is_equal,
                    )
                    oh_g = pog.tile([128, E, 8], bf16, tag="oh")
                    nc.vector.tensor_copy(oh_g[:], ohu_g[:])

                    # counts per (e,b): [64,1] = oh^T @ ones
                    oh_flat = oh_g[:].rearrange("p e b -> p (e b)")
                    cntp = pcnt.tile([64, 1], f32, tag="cntoff")
                    nc.tensor.matmul(
                        out=cntp[:], lhsT=oh_flat, rhs=ones128bf[:],
                        start=True, stop=True,
                    )
                    cnt_sb = psmA.tile([64, 1], f32, tag="cntsb")
                    nc.vector.tensor_copy(cnt_sb[:], cntp[:])
                    # intra-group exclusive prefix over blocks (col 8: totals)
                    offp = pcnt.tile([1, 72], f32, tag="cntoff")
                    nc.tensor.matmul(
                        out=offp[:], lhsT=cnt_sb[:], rhs=tsel_sb[:],
                        start=True, stop=True,
                    )
                    offs_sb = poffs.tile([1, E, 9], f32, tag="offs")
                    nc.vector.tensor_tensor(
                        offs_sb[:], offp[:].rearrange("p (e b) -> p e b", e=8),
                        run_row[:].unsqueeze(2).to_broadcast([1, E, 9]),
                        op=OP.add,
                    )
                    run_next = prun.tile([1, E], f32, tag="run")
                    nc.vector.tensor_copy(
                        run_next[:].unsqueeze(2), offs_sb[:, :, 8:9]
                    )
                    run_row = run_next

                    # dest = rank-in-block + group offset, per (token, e, b)
                    pos = ppos.tile([128, E, 8], f32, tag="pos")
                    nc.tensor.matmul(
                        out=pos[:].rearrange("p e b -> p (e b)"), lhsT=cum_sb[:],
                        rhs=oh_flat, start=True, stop=False, skip_group_check=True,
                    )
                    nc.tensor.matmul(
                        out=pos[:].rearrange("p e b -> p (e b)"), lhsT=onesb1[:],
                        rhs=offs_sb[:, :, 0:8], start=False, stop=True,
                        skip_group_check=True,
                    )
                    seld = psmA.tile([128, E, 8], f32, tag="seld")
                    nc.vector.select(
                        seld[:], ohu_g[:], pos[:],
                        zeros64_sb[:].rearrange("p (e b) -> p e b", e=8),
                    )
                    destf = psmA.tile([128, 8], f32, tag="destf")
                    nc.vector.tensor_reduce(
                        destf[:], seld[:].transpose([0, 2, 1]), axis=AX.X, op=OP.add
                    )
                    dest_tk = dest_all[:].rearrange("p (t k) -> p t k", k=2)
                    nc.vector.tensor_copy(
                        dest_tk[:, 4 * g: 4 * g + 4, 0:1],
                        destf[:, 0:4].unsqueeze(2),
                    )
                    nc.vector.tensor_copy(
                        dest_tk[:, 4 * g: 4 * g + 4, 1:2],
                        destf[:, 4:8].unsqueeze(2),
                    )
                    for b in range(8):
                        col = (4 * g + (b % 4)) * 2 + (b // 4)
                        nc.gpsimd.indirect_dma_start(
                            out=g_dram[:],
                            out_offset=bass.IndirectOffsetOnAxis(
                                ap=dest_all[:, col: col + 1], axis=0
                            ),
                            in_=xn8_all[:, 4 * g + (b % 4), :],
                            in_offset=None,
                        )

                # batched gates
                nc.scalar.activation(w0_all[:], gap_all[:], AF.Sigmoid, scale=SW)
                nc.vector.tensor_tensor(
                    w1_all[:], ones_col[:].to_broadcast([128, NT]), w0_all[:],
                    op=OP.subtract,
                )

            # ---------------- Phase E: expert FFN loop
            with (
                tc.tile_pool(name="pgr", bufs=3) as pgr,
                tc.tile_pool(name="pgts", bufs=2) as pgts,
                tc.tile_pool(name="pwu", bufs=2) as pwu,
                tc.tile_pool(name="pwd", bufs=2) as pwd,
                tc.tile_pool(name="pht", bufs=1) as pht,
                tc.tile_pool(name="psil", bufs=2) as psil,
                tc.tile_pool(name="psilt", bufs=2) as psilt,
                tc.tile_pool(name="pdo", bufs=3) as pdo,
                tc.tile_pool(name="ppmain", bufs=2, space="PSUM") as ppmain,
                tc.tile_pool(name="pptail", bufs=2, space="PSUM") as pptail,
                tc.tile_pool(name="ppd", bufs=2, space="PSUM") as ppd,
            ):
                for e in range(E):
                    wu_sb = pwu.tile([128, 4, 2, 2 * H], f8, tag="wu")
                    nc.sync.dma_start(
                        out=wu_sb[:].rearrange("p dp kt c -> p (dp kt c)"),
                        in_=wu_d[e],
                    )
                    wd_sb = pwd.tile([128, 8, 2, D], f8, tag="wd")
                    nc.sync.dma_start(
                        out=wd_sb[:].rearrange("p q kt d -> p (q kt d)"), in_=wd_d[e]
                    )

                    # gather fp8 rows + transpose to [d, rows]
                    gts = pgts.tile([128, 8, ceff], f8, tag="gts")
                    for rt in range(RT):
                        w = min(128, ceff - rt * 128)
                        gr = pgr.tile([128, D], f8, tag="gr")
                        nc.sync.dma_start(
                            out=gr[:],
                            in_=g_dram[
                                e * CCAP + rt * 128: e * CCAP + (rt + 1) * 128, :
                            ],
                        )
                        for half in range(2):
                            # fp8 transpose hw mode writes with element step 2
                            trb = ppmain.tile([128, 4, 128, 2], f8, tag="pug")
                            for q in range(4):
                                dc = half * 4 + q
                                nc.tensor.transpose(
                                    trb[:, q, :, 0],
                                    gr[:, dc * 128:(dc + 1) * 128],
                                    ident8_sb[:],
                                )
                            nc.vector.tensor_copy(
                                gts[:, half * 4:(half + 1) * 4,
                                    rt * 128: rt * 128 + w],
                                trb[:, :, 0:w, 0],
                            )

                    # up-GEMM (fp8 DoubleRow, h^T layout) + SwiGLU via Silu
                    hts = pht.tile([128, 16, ceff], f8, tag="ht")
                    for hg in range(16):
                        pug = ppmain.tile([128, 1024], f32, tag="pug")
                        pugt = pptail.tile([128, 2 * CT], f32, tag="pugt")
                        for part in range(2):
                            col0 = hg * 128 + part * H
                            for dp in range(4):
                                nc.tensor.matmul(
                                    out=pug[:, part * 512:(part + 1) * 512],
                                    lhsT=wu_sb[:, dp, :, col0:col0 + 128],
                                    rhs=gts[:, 2 * dp:2 * dp + 2, 0:512],
                                    start=(dp == 0), stop=(dp == 3), perf_mode=DR,
                                )
                            for dp in range(4):
                                nc.tensor.matmul(
                                    out=pugt[:, part * CT:(part + 1) * CT],
                                    lhsT=wu_sb[:, dp, :, col0:col0 + 128],
                                    rhs=gts[:, 2 * dp:2 * dp + 2, 512:ceff],
                                    start=(dp == 0), stop=(dp == 3), perf_mode=DR,
                                )
                        for (pt, po, cs, w, pp) in (
                            (pug, 512, slice(0, 512), 512, psil),
                            (pugt, CT, slice(512, ceff), CT, psilt),
                        ):
                            sil = pp.tile([128, w], f32, tag=f"sil{w}")
                            if use_silu:
                                nc.scalar.activation(
                                    sil[:], pt[:, po:2 * po], AF.Silu
                                )
                                nc.vector.tensor_tensor(
                                    hts[:, hg, cs], pt[:, 0:po], sil[:],
                                    op=OP.mult,
                                )
                            else:
                                nc.scalar.activation(
                                    sil[:], pt[:, po:2 * po], AF.Sigmoid
                                )
                                h1 = pp.tile([128, w], f32, tag=f"h1{w}")
                                nc.vector.tensor_tensor(
                                    h1[:], pt[:, 0:po], sil[:], op=OP.mult
                                )
                                nc.vector.tensor_tensor(
                                    hts[:, hg, cs], h1[:], pt[:, po:2 * po],
                                    op=OP.mult,
                                )

                    # down-GEMM (fp8 DR)
                    for rt in range(RT):
                        m = min(128, ceff - rt * 128)
                        rs = slice(rt * 128, rt * 128 + m)
                        do = pdo.tile([128, D], f16, tag="do")
                        for dq in range(2):
                            pd = ppd.tile([128, 512], f32, tag="pd")
                            for q in range(8):
                                nc.tensor.matmul(
                                    out=pd[0:m, :],
                                    lhsT=hts[:, 2 * q:2 * q + 2, rs],
                                    rhs=wd_sb[:, q, :, dq * 512:(dq + 1) * 512],
                                    start=(q == 0), stop=(q == 7), perf_mode=DR,
                                )
                            nc.scalar.copy(
                                do[0:m, dq * 512:(dq + 1) * 512], pd[0:m, :]
                            )
                        nc.sync.dma_start(
                            out=dn_dram[e * CCAP + rt * 128: e * CCAP + rt * 128 + m, :],
                            in_=do[0:m, :],
                        )

            # ---------------- Phase F: combine
            with (
                tc.tile_pool(name="pgd", bufs=6) as pgd,
                tc.tile_pool(name="pcmb", bufs=3) as pcmb,
            ):
                for i in range(NT):
                    ts = slice(i * 128, (i + 1) * 128)
                    g01 = pgd.tile([128, 2, D], f16, tag="gd")
                    if batched_gather:
                        nc.gpsimd.indirect_dma_start(
                            out=g01[:],
                            out_offset=None,
                            in_=dn_dram[:],
                            in_offset=bass.IndirectOffsetOnAxis(
                                ap=dest_all[:, 2 * i: 2 * i + 2], axis=0
                            ),
                        )
                    else:
                        for k in range(2):
                            nc.gpsimd.indirect_dma_start(
                                out=g01[:, k, :],
                                out_offset=None,
                                in_=dn_dram[:],
                                in_offset=bass.IndirectOffsetOnAxis(
                                    ap=dest_all[:, 2 * i + k: 2 * i + k + 1],
                                    axis=0,
                                ),
                            )
                    acc = pcmb.tile([128, D], f32, tag="acc")
                    nc.vector.scalar_tensor_tensor(
                        out=acc[:], in0=g01[:, 0, :], scalar=w0_all[:, i: i + 1],
                        in1=xbf_all[:, i, :], op0=OP.mult, op1=OP.add,
                    )
                    outt = pcmb.tile([128, D], f32, tag="out")
                    nc.vector.scalar_tensor_tensor(
                        out=outt[:], in0=g01[:, 1, :], scalar=w1_all[:, i: i + 1],
                        in1=acc[:], op0=OP.mult, op1=OP.add,
                    )
                    nc.sync.dma_start(out=out_d[ts, :], in_=outt[:])

    if strip_waw:
        _strip_scatter_waw(nc)
    if split_waits:
        _split_excess_waits(nc)
    return nc


def host_prep(x, norm_scale, w_router, w_up, w_down):
    """Shard x, fold norm_scale into router/up weights, build layouts."""
    x = np.asarray(x, dtype=np.float32)
    norm_scale = np.asarray(norm_scale, dtype=np.float32)
    w_router = np.asarray(w_router, dtype=np.float32)
    w_up = np.asarray(w_up, dtype=np.float32)
    w_down = np.asarray(w_down, dtype=np.float32)

    tokens = x.reshape(-1, D)
    shards = [
        np.ascontiguousarray(tokens[c * T_PER_CORE:(c + 1) * T_PER_CORE])
        for c in range(N_CORES)
    ]

    # router: [p, dc*8+e] = (w_router*ns).T[dc*128+p, e]
    wrT = (w_router * norm_scale[None, :]).T  # [D, E]
    wr = np.ascontiguousarray(
        wrT.reshape(8, 128, E).transpose(1, 0, 2).reshape(128, 8 * E)
    )

    # up: wuT[e, d, col] with cols = [u | g] -> [e, p, dp, kt, 2H] fp8 * SW
    wuT = (w_up * norm_scale[None, None, :]).transpose(0, 2, 1)  # [E, D, 2H]
    wu8 = np.ascontiguousarray(
        (wuT * SW).reshape(E, 4, 2, 128, 2 * H)
        .transpose(0, 3, 1, 2, 4)
        .reshape(E, 128, 8 * 2 * H)
    ).astype(ml_dtypes.float8_e4m3fn)

    # down: wdT[e, h, d]; [e, p, q, kt, d] = wdT[e, (q*2+kt)*128+p, d] fp8
    wdT = w_down.transpose(0, 2, 1)  # [E, H, D]
    wd8 = np.ascontiguousarray(
        wdT.reshape(E, 8, 2, 128, D).transpose(0, 3, 1, 2, 4)
        .reshape(E, 128, 8 * 2 * D)
    ).astype(ml_dtypes.float8_e4m3fn)

    ident8 = np.eye(128).astype(ml_dtypes.float8_e4m3fn)
    identf8 = np.eye(8, dtype=np.float32)
    cum = np.triu(np.ones((128, 128)), k=1).astype(ml_dtypes.bfloat16)
    # tsel[(e',b'), (e,bb)] = (e'==e) & (b' < bb), bb in 0..8 (8 = totals)
    ep, bp = np.meshgrid(np.arange(E), np.arange(8), indexing="ij")
    epf = ep.reshape(-1)
    bpf = bp.reshape(-1)
    e2, bb = np.meshgrid(np.arange(E), np.arange(9), indexing="ij")
    e2f = e2.reshape(-1)
    bbf = bb.reshape(-1)
    tsel = ((epf[:, None] == e2f[None, :]) & (bpf[:, None] < bbf[None, :])).astype(
        np.float32
    )
    iota_ie = np.tile(np.arange(E, dtype=np.float32), (128, 4)).reshape(128, 32)
    iota_ei = np.tile(
        np.repeat(np.arange(E, dtype=np.float32), 4), (128, 1)
    ).reshape(128, 32)
    base8 = (np.arange(E, dtype=np.float32) * CCAP).reshape(1, E)

    common = {
        "wr": wr,
        "wu": wu8,
        "wd": wd8,
        "ident8": ident8,
        "identf8": identf8,
        "cum": cum,
        "tsel": tsel,
        "iota_ie": iota_ie,
        "iota_ei": iota_ei,
        "base8": base8,
    }
    in_maps = []
    for c in range(N_CORES):
        sh = shards[c]
        xT = np.ascontiguousarray(
            sh.T.reshape(8, 128, T_PER_CORE).transpose(1, 0, 2)
            .reshape(128, 8 * T_PER_CORE)
        )
        in_maps.append(
            {
                "xbf": sh.astype(ml_dtypes.bfloat16),
                "xT": xT,
                **common,
            }
        )
    return in_maps


def _max_group_count(x, norm_scale, w_router):
    """Host-side routing replication to validate the capacity CEFF."""
    tokens = np.asarray(x, dtype=np.float32).reshape(-1, D)
    wrT = (np.asarray(w_router, dtype=np.float32)
           * np.asarray(norm_scale, dtype=np.float32)[None, :]).T
    mx = 0
    for c in range(N_CORES):
        sc = tokens[c * T_PER_CORE:(c + 1) * T_PER_CORE] @ wrT
        top1 = np.argmax(sc, axis=1)
        sc2 = sc.copy()
        sc2[np.arange(len(sc2)), top1] = -np.inf
        top2 = np.argmax(sc2, axis=1)
        cnts = np.bincount(top1, minlength=E) + np.bincount(top2, minlength=E)
        mx = max(mx, int(cnts.max()))
    return mx


def kernel(x, norm_scale, w_router, w_up, w_down):
    from concourse.bass_utils import run_bass_kernel_spmd

    ceff = CEFF
    mx = _max_group_count(x, norm_scale, w_router)
    if mx > CEFF - 4:
        ceff = CCAP  # fallback: full capacity (uneven inputs)
    key = ("nc", ceff)
    if key not in _CACHE:
        _CACHE[key] = build_program(ceff=ceff, batched_gather=False)
    nc = _CACHE[key]

    in_maps = host_prep(x, norm_scale, w_router, w_up, w_down)
    res = run_bass_kernel_spmd(nc, in_maps, core_ids=list(range(N_CORES)))
    out = np.concatenate([res.results[c]["out"] for c in range(N_CORES)], axis=0)
    return out.reshape(np.asarray(x).shape).astype(np.float32)
